# revision 29
# baseline (speedup 1.0000x reference)
"""Cross-covariance-style attention (XCA variant, no q/k transpose) on 8 TRN2 cores.

Reference computation (per batch element b, H=8 heads, hd=96):
    qkv = x @ w_qkv                      # [N=1024, 3C], C=768
    q, k, v = split(qkv)                 # each [H, N, hd] logically
    qn = q / ||q||_row;  kn = k / ||k||_row
    S = (qn @ kn^T) * temperature        # [H, N, N]
    P = softmax(S, axis=-1)
    out = P @ v                          # [H, N, hd]
    y = out @ w_proj + b_proj            # [N, C]

Sharding: data-parallel over batch B=8 -> one batch element per NeuronCore,
no collectives.  Each core runs the identical program on its slice.

Per-core dataflow (all matmuls bf16 except the final projection in f32r):
  - xT loaded via DMA-transpose (bf16), w_qkv loaded bf16.
  - qkv matmul accumulates in PSUM f32; row sum-of-squares computed straight
    from PSUM (DVE), sqrt on ACT, reciprocal on DVE.
  - q is scaled by 1/||q|| while copying PSUM->SBUF; k copied unscaled and
    temperature/||k|| folded into the Exp activation's per-partition scale.
  - S^T = k @ qn^T computed per head with d on partitions (q,k DMA-transposed
    per head; head dim padded 96->128 to satisfy the XBAR 128-col rule).
  - P^T = exp(scale * S^T) written bf16 by ACT directly from PSUM.
  - out^T = [v | 1]^T-style matmul: lhsT = v with a ones column appended, so
    PSUM row 96 accumulates the softmax denominator for free.
  - denominator reciprocal -> DMA partition-broadcast -> DVE multiply produces
    the normalized out^T in f32.
  - projection runs in float32r (full PE rate at free-dim>=256) with K=96
    per-head accumulation; bias added from a broadcast tile on DVE.
"""

import os

import numpy as np
import ml_dtypes

import concourse.bass as bass
import concourse.tile as tile
import concourse.mybir as mybir
from concourse.vector_clock import ScopedClock
from concourse.bass_utils import run_bass_kernel_spmd

B, N, C = 8, 1024, 768
H, HD = 8, 96
NM = N // 128          # 8 row chunks of 128
KC = C // 128          # 6 contraction chunks for qkv
NB = 384               # qkv output column chunk (aligns q/k/v and head bounds)
NQKV = (3 * C) // NB   # 6 column chunks: 0,1=q 2,3=k 4,5=v
F32 = mybir.dt.float32
F32R = mybir.dt.float32r
BF16 = mybir.dt.bfloat16
MULT = mybir.AluOpType.mult
ADD = mybir.AluOpType.add


class SafeTileContext(tile.TileContext):
    """This toolchain's walrus rejects >1 sync wait on the final Drain; split
    the end-of-context quiesce waits across single-wait nops."""

    MAXW = 1

    def _drain_and_barrier(self, tick_clock, wait_clock):
        nc = self.nc
        drain_inst = nc.sync.drain()
        wait_clock.add_sem_waits(
            drain_inst.ins, ScopedClock({None: tick_clock.global_clock})
        )
        si = drain_inst.ins.sync_info
        waits = list(si.on_wait or [])
        if len(waits) > self.MAXW:
            si.on_wait = waits[: self.MAXW]
            rest = waits[self.MAXW :]
            for i in range(0, len(rest), self.MAXW):
                nop = nc.sync.nop()
                nsi = nop.ins.sync_info
                chunk = rest[i : i + self.MAXW]
                if nsi is None:
                    nop.ins.sync_info = mybir.SyncInfo(on_wait=chunk, on_update=[])
                else:
                    nsi.on_wait = list(nsi.on_wait or []) + chunk
                    nop.ins.sync_info = nsi
        nc.all_engine_barrier()
        popped = nc._tile_sem_poison_stack.pop()
        assert popped is self._sem_poison
        # clear_and_free_semaphores uses EVENT_SEMAPHORE_RANGE_CLEAR, which
        # this walrus can't encode.  Reset each semaphore to zero with a
        # sem-wr-imm EventSemaphore on the gpsimd engine instead.
        sems = list(self.sems.allocated().values())
        if sems:
            sem_nums = [s.num if hasattr(s, "num") else int(s) for s in sems]
            for i, num in enumerate(sem_nums):
                inst = mybir.InstEventSemaphore(
                    name=f"semwr-{num}-{i}", ins=[], outs=[]
                )
                inst.engine = mybir.EngineType.Pool
                inst.sync_info = mybir.SyncInfo(
                    on_wait=[],
                    on_update=[
                        mybir.SyncUpdate(
                            id=num, sync_type="semaphore",
                            update_mode="sem-wr-imm", update_value=0,
                        )
                    ],
                )
                nc.register_instruction(inst)
                nc.cur_bb.bb.add_instruction(inst)
            nc._state.prepend_free_semaphores(sem_nums)
            for poison_set in nc._tile_sem_poison_stack:
                poison_set.update(sem_nums)
        nc.all_engine_barrier()


def _split_multi_waits(nc):
    """This toolchain's walrus encodes at most ONE sync wait per instruction.
    Hoist extra waits onto same-engine InstNoOp's inserted just before the
    offending instruction (the engine executes its stream in order, so the
    quiesce semantics are identical)."""
    counter = 0
    for f in nc.m.functions:
        for bb in f.blocks:
            insts = list(bb.instructions)
            out = []
            changed = False
            for inst in insts:
                si = inst.sync_info
                waits = list(si.on_wait) if si and si.on_wait else []
                if len(waits) > 1 and inst.engine != mybir.EngineType.Unassigned:
                    for w in waits[:-1]:
                        counter += 1
                        nop = mybir.InstNoOp(name=f"swsplit-{counter}", ins=[], outs=[])
                        nop.engine = inst.engine
                        nop.sync_info = mybir.SyncInfo(on_wait=[w], on_update=[])
                        nc.register_instruction(nop)
                        out.append(nop)
                    si.on_wait = [waits[-1]]
                    inst.sync_info = si
                    changed = True
                out.append(inst)
            if changed:
                bb.instructions = out
    return nc


def build():
    nc = bass.Bass("TRN2")
    x = nc.dram_tensor("x", [N, C], BF16, kind="ExternalInput")
    w_qkv = nc.dram_tensor("w_qkv", [C, 3 * C], BF16, kind="ExternalInput")
    temp = nc.dram_tensor("temperature", [H], F32, kind="ExternalInput")
    w_proj = nc.dram_tensor("w_proj", [C, C], BF16, kind="ExternalInput")
    b_proj = nc.dram_tensor("b_proj", [C], F32, kind="ExternalInput")
    y = nc.dram_tensor("y", [N, C], F32, kind="ExternalOutput")

    wq_t = w_qkv.rearrange("(k p) n -> k p n", p=128)   # [6, 128, 2304]
    wp_t = w_proj.rearrange("(h d) j -> h d j", d=HD)   # [8, 96, 768]

    with SafeTileContext(nc) as tc:
        with tc.tile_pool(name="persist", bufs=1) as pp, \
             tc.tile_pool(name="small", bufs=1) as sp:
            # ---- constants / weights that live through the whole kernel ----
            temp_b = sp.tile([128, H], F32, name="temp_b")
            temp_ap = temp[:]
            nc.gpsimd.dma_start(
                out=temp_b,
                in_=bass.AP(
                    tensor=temp_ap.tensor, offset=temp_ap.offset,
                    ap=[[0, 128]] + list(temp_ap.ap),
                ),
            )
            b_bcast = sp.tile([128, C], F32, name="b_bcast")
            bp_ap = b_proj[:]
            nc.gpsimd.dma_start(
                out=b_bcast,
                in_=bass.AP(
                    tensor=bp_ap.tensor, offset=bp_ap.offset,
                    ap=[[0, 128]] + list(bp_ap.ap),
                ),
            )
            wproj_sb = []
            for h in range(H):
                t = pp.tile([HD, C], BF16, name=f"wp{h}")
                nc.scalar.dma_start(out=t, in_=wp_t[h])
                wproj_sb.append(t)

            # ---- persistent activation tensors ----
            q_sb = [pp.tile([128, H, 128], BF16, name=f"q{m}") for m in range(NM)]
            k_sb = [pp.tile([128, H, 128], BF16, name=f"k{m}") for m in range(NM)]
            vext = [pp.tile([128, H, HD + 1], BF16, name=f"v{m}") for m in range(NM)]
            rkt = [sp.tile([128, H], F32, name=f"rkt{m}") for m in range(NM)]
            rr = [sp.tile([128, 16], F32, name=f"rr{m}") for m in range(NM)]
            rks = [sp.tile([128, 16], F32, name=f"rks{m}") for m in range(NM)]
            outT = [pp.tile([HD, N], BF16, name=f"oT{h}") for h in range(H)]

            for m in range(NM):
                nc.vector.memset(q_sb[m][:, :, HD:], 0.0)
                nc.vector.memset(k_sb[m][:, :, HD:], 0.0)
                nc.vector.memset(vext[m][:, :, HD:], 1.0)


            # ================= phase 1: qkv projection + norms =================
            with tc.tile_pool(name="wqkv", bufs=1) as wqp, \
                 tc.tile_pool(name="xT", bufs=1) as xtp, \
                 tc.tile_pool(name="qkv_ps", bufs=4, space="PSUM") as qkvp, \
                 tc.tile_pool(name="scr", bufs=2) as scrp:
                wqkv_sb = []
                for kk in range(KC):
                    t = wqp.tile([128, 3 * C], BF16, name=f"wq{kk}")
                    nc.sync.dma_start(out=t, in_=wq_t[kk])
                    wqkv_sb.append(t)
                xT_sb = []
                for kk in range(KC):
                    t = xtp.tile([128, N], BF16, name=f"xT{kk}")
                    nc.sync.dma_start(
                        out=t, in_=x[:, kk * 128 : (kk + 1) * 128], transpose=True
                    )
                    xT_sb.append(t)

                for nb in range(NQKV):  # 0,1=q  2,3=k  4,5=v
                    for m in range(NM):
                        ps = qkvp.tile([128, NB], F32, name="qkv")
                        for kk in range(KC):
                            nc.tensor.matmul(
                                ps,
                                lhsT=xT_sb[kk][:, m * 128 : (m + 1) * 128],
                                rhs=wqkv_sb[kk][:, nb * NB : (nb + 1) * NB],
                                start=(kk == 0),
                                stop=(kk == KC - 1),
                            )
                        if nb < 2:          # ---- q: normalize while copying
                            qtmps = []
                            for hh in range(4):
                                h = nb * 4 + hh
                                sl = ps[:, hh * HD : (hh + 1) * HD]
                                qtmp = scrp.tile([128, HD], F32, name=f"qtmp{hh}")
                                nc.vector.tensor_copy(out=qtmp, in_=sl)
                                qtmps.append(qtmp)
                                scr = scrp.tile([128, HD], F32, name="scr")
                                nc.vector.tensor_mul(out=scr, in0=qtmp, in1=qtmp)
                                nc.vector.reduce_sum(
                                    out=rks[m][:, h : h + 1], in_=scr,
                                    axis=mybir.AxisListType.X,
                                )
                            nc.scalar.activation(
                                out=rr[m][:, nb * 4 : nb * 4 + 4],
                                in_=rks[m][:, nb * 4 : nb * 4 + 4],
                                func=mybir.ActivationFunctionType.Sqrt,
                            )
                            nc.vector.reciprocal(
                                out=rr[m][:, nb * 4 : nb * 4 + 4],
                                in_=rr[m][:, nb * 4 : nb * 4 + 4],
                            )
                            for hh in range(4):
                                h = nb * 4 + hh
                                nc.vector.tensor_scalar_mul(
                                    out=q_sb[m][:, h, :HD],
                                    in0=qtmps[hh],
                                    scalar1=rr[m][:, h : h + 1],
                                )
                        elif nb < 4:        # ---- k: copy raw, rkt = temp/||k||
                            for hh in range(4):
                                h = (nb - 2) * 4 + hh
                                sl = ps[:, hh * HD : (hh + 1) * HD]
                                nc.vector.tensor_copy(
                                    out=k_sb[m][:, h, :HD], in_=sl
                                )
                                scr = scrp.tile([128, HD], F32, name="scr")
                                nc.vector.tensor_mul(
                                    out=scr,
                                    in0=k_sb[m][:, h, :HD],
                                    in1=k_sb[m][:, h, :HD],
                                )
                                nc.vector.reduce_sum(
                                    out=rks[m][:, 8 + h : 9 + h], in_=scr,
                                    axis=mybir.AxisListType.X,
                                )
                            c0 = 8 + (nb - 2) * 4
                            nc.scalar.activation(
                                out=rr[m][:, c0 : c0 + 4],
                                in_=rks[m][:, c0 : c0 + 4],
                                func=mybir.ActivationFunctionType.Sqrt,
                            )
                            nc.vector.reciprocal(
                                out=rr[m][:, c0 : c0 + 4], in_=rr[m][:, c0 : c0 + 4]
                            )
                            h0 = (nb - 2) * 4
                            nc.vector.tensor_mul(
                                out=rkt[m][:, h0 : h0 + 4],
                                in0=rr[m][:, c0 : c0 + 4],
                                in1=temp_b[:, h0 : h0 + 4],
                            )
                        else:               # ---- v: copy into [v | pad-ones]
                            for hh in range(4):
                                h = (nb - 4) * 4 + hh
                                nc.vector.tensor_copy(
                                    out=vext[m][:, h, :HD],
                                    in_=ps[:, hh * HD : (hh + 1) * HD],
                                )

            # ================= phase 2: attention per head =================
            with tc.tile_pool(name="qkT", bufs=2) as qktp, \
                 tc.tile_pool(name="pT", bufs=2) as ptp, \
                 tc.tile_pool(name="dn", bufs=2) as dnp, \
                 tc.tile_pool(name="s_ps", bufs=4, space="PSUM") as spp, \
                 tc.tile_pool(name="o_ps", bufs=3, space="PSUM") as opp, \
                 tc.tile_pool(name="dscr", bufs=2, space="DRAM") as dsp:
                for h in range(H):
                    qT = qktp.tile([128, N], BF16, name="qT")
                    kT = qktp.tile([128, N], BF16, name="kT")
                    for m in range(NM):
                        nc.sync.dma_start(
                            out=qT[:, m * 128 : (m + 1) * 128],
                            in_=q_sb[m][:, h, :], transpose=True,
                        )
                        nc.sync.dma_start(
                            out=kT[:, m * 128 : (m + 1) * 128],
                            in_=k_sb[m][:, h, :], transpose=True,
                        )
                    pTs = []
                    for m in range(NM):
                        pTm = ptp.tile([128, N], BF16, name=f"pT{m}")
                        for nb2 in range(2):
                            ps = spp.tile([128, 512], F32, name="s")
                            nc.tensor.matmul(
                                ps,
                                lhsT=kT[:HD, m * 128 : (m + 1) * 128],
                                rhs=qT[:HD, nb2 * 512 : (nb2 + 1) * 512],
                                start=True, stop=True,
                            )
                            nc.scalar.activation(
                                out=pTm[:, nb2 * 512 : (nb2 + 1) * 512],
                                in_=ps,
                                func=mybir.ActivationFunctionType.Exp,
                                scale=rkt[m][:, h : h + 1],
                            )
                        pTs.append(pTm)
                    for nb2 in range(2):
                        po = opp.tile([HD + 1, 512], F32, name="po")
                        for m in range(NM):
                            nc.tensor.matmul(
                                po,
                                lhsT=vext[m][:, h, :],
                                rhs=pTs[m][:, nb2 * 512 : (nb2 + 1) * 512],
                                start=(m == 0),
                                stop=(m == NM - 1),
                            )
                        den = dnp.tile([HD + 1, 512], F32, name="den")
                        nc.vector.tensor_copy(
                            out=den[HD : HD + 1, :], in_=po[HD : HD + 1, :]
                        )
                        nc.vector.reciprocal(
                            out=den[HD : HD + 1, :], in_=den[HD : HD + 1, :]
                        )
                        # broadcast 1/denom across the 96 head-dim partitions
                        # via a DRAM bounce (DMA can replicate partitions)
                        dscr = dsp.tile([1, 512], F32, name="dscr")
                        nc.sync.dma_start(out=dscr, in_=den[HD : HD + 1, :])
                        rb = dnp.tile([HD, 512], F32, name="rb")
                        nc.gpsimd.dma_start(
                            out=rb,
                            in_=bass.AP(
                                tensor=dscr.tensor, offset=dscr.offset,
                                ap=[[0, HD]] + list(dscr.ap)[1:],
                            ),
                        )
                        nc.vector.tensor_mul(
                            out=outT[h][:, nb2 * 512 : (nb2 + 1) * 512],
                            in0=po[:HD, :],
                            in1=rb,
                        )

            # ================= phase 3: projection + bias =================
            with tc.tile_pool(name="y_ps", bufs=3, space="PSUM") as ypp, \
                 tc.tile_pool(name="ysb", bufs=2) as ysp:
                for m in range(NM):
                    ym = ysp.tile([128, C], F32, name="ym")
                    for jb in range(2):
                        py = ypp.tile([128, NB], F32, name="py")
                        for h in range(H):
                            nc.tensor.matmul(
                                py,
                                lhsT=outT[h][:, m * 128 : (m + 1) * 128],
                                rhs=wproj_sb[h][:, jb * NB : (jb + 1) * NB],
                                start=(h == 0),
                                stop=(h == H - 1),
                            )
                        nc.vector.tensor_add(
                            out=ym[:, jb * NB : (jb + 1) * NB],
                            in0=py,
                            in1=b_bcast[:, jb * NB : (jb + 1) * NB],
                        )
                    nc.scalar.dma_start(
                        out=y[m * 128 : (m + 1) * 128, :], in_=ym
                    )
    return _split_multi_waits(nc)


_NC = None
LAST_RESULT = None


def kernel(x, w_qkv, temperature, w_proj, b_proj):
    global _NC, LAST_RESULT
    if _NC is None:
        _NC = build()
    xb = np.asarray(x, dtype=np.float32).astype(ml_dtypes.bfloat16)
    wqb = np.asarray(w_qkv, dtype=np.float32).astype(ml_dtypes.bfloat16)
    tf = np.ascontiguousarray(np.asarray(temperature, dtype=np.float32).reshape(H))
    wp = np.asarray(w_proj, dtype=np.float32).astype(ml_dtypes.bfloat16)
    bp = np.ascontiguousarray(np.asarray(b_proj, dtype=np.float32))
    in_maps = [
        {
            "x": np.ascontiguousarray(xb[i]),
            "w_qkv": np.ascontiguousarray(wqb),
            "temperature": tf,
            "w_proj": wp,
            "b_proj": bp,
        }
        for i in range(B)
    ]
    trace = bool(int(os.environ.get("KERNEL_TRACE", "0")))
    res = run_bass_kernel_spmd(
        _NC, in_maps, core_ids=list(range(B)), trace=trace
    )
    LAST_RESULT = res
    out = np.stack([res.results[i]["y"] for i in range(B)], axis=0)
    return out.astype(np.float32)
